# revision 27
# baseline (speedup 1.0000x reference)
"""GAT (2-layer, DGL-style) on 8 Trainium2 NeuronCores — v2.

Strategy
--------
- Shard destination nodes (2500/core, padded to 2560 = 20 windows x 128).
  Each core owns all edges pointing at its nodes, sorted by dst.
- Phase A is node-major: feat[window] = xT_chunk^T @ W1 directly (bf16),
  with al/ar folded into 8 extra W columns on the host, so el/er drop out
  of the same stationary loads (no PE transposes, no elr matmuls).
- Node tables (feat | el) are AllGathered in 2 chunks (local nodes
  0:1280 / 1280:2560) into two Shared tensors, so each collective starts
  as soon as its half of phase A is done and overlaps the rest.
- Edge slots within each window are grouped by source chunk (each group
  padded to 128-slot blocks) so the per-edge row fetch is one dma_gather
  per (window, chunk) against that chunk's table.
- Softmax numerator ex = exp(leaky_relu(el_src + er_dst)) without the
  max-shift (range-safe here).
- exp and the broadcast head->feature expansion are fused into one ACT
  op per window (0-stride input view, dense output), so the msg multiply
  is a dense 2x-mode tensor_tensor; value aggregation via one-hot
  selector matmuls S^T @ msg into PSUM; denominators via a flipped
  matmul (stationary = ex columns, streaming S) accumulated across
  blocks, transposed once per window.
- Gather slots are sorted by table row id (HBM-friendly monotonic
  descriptor streams) and issued as <=4-block single_packet calls so the
  SWDGE ring never backs up the POOL engine.
"""

import os
import numpy as np
import ml_dtypes

import concourse.bass as bass
import concourse.bacc as bacc
import concourse.tile as tile
import concourse.mybir as mybir
from concourse.bass_utils import run_bass_kernel_spmd

F32 = mybir.dt.float32
BF16 = mybir.dt.bfloat16
F8 = mybir.dt.float8e4
I16 = mybir.dt.int16
AF = mybir.ActivationFunctionType
OP = mybir.AluOpType

NP_F8 = ml_dtypes.float8_e4m3

# problem constants (fixed by the harness)
N, E, IN_DIM, HID, OUT = 20000, 320000, 256, 128, 64
H0, D0 = 4, 128
F0 = H0 * D0            # 512
NCORES = 8
NS = N // NCORES        # 2500 owned nodes per core
WSZ = 128
NW = 20                 # windows per core (20*128 = 2560 >= 2500)
NSP = NW * WSZ          # 2560 padded local rows
NCH = 2                 # AllGather chunks (local nodes 0:1280 / 1280:2560)
CH = NSP // NCH         # 1280 local rows per chunk
NGC = NCORES * CH       # 10240 rows per chunk table

FE1_W = 640             # bf16 row: feat 0:512, el 512:516, pad -> 1280 B
FE2_W = 128             # bf16 row: feat2 0:64, el2 64:65, pad -> 256 B


# --------------------------------------------------------------------------
# host-side graph prep (index/layout work only)
# --------------------------------------------------------------------------

def _pack_idx(ids: np.ndarray) -> np.ndarray:
    """Pack an index list for dma_gather: position i -> partition i%16,
    col i//16, replicated across the 8 groups of 16 partitions."""
    n = ids.shape[0]
    assert n % 16 == 0
    t = ids.reshape(n // 16, 16).T.astype(np.int16)     # [16, n//16]
    return np.tile(t, (8, 1))                            # [128, n//16]


def _host_prep(src, dst):
    src = np.asarray(src).astype(np.int64)
    dst = np.asarray(dst).astype(np.int64)
    order = np.argsort(dst, kind="stable")
    src, dst = src[order], dst[order]

    cores = []
    # shared per-(window, chunk) block counts (max over cores)
    nbc = np.zeros((NW, NCH), np.int64)
    for c in range(NCORES):
        m = (dst // NS) == c
        src_c, dl = src[m], dst[m] - c * NS
        w_of = dl // WSZ
        ch_of = (src_c % NS) // CH
        for w in range(NW):
            for ch in range(NCH):
                cnt = int(((w_of == w) & (ch_of == ch)).sum())
                nbc[w, ch] = max(nbc[w, ch], (cnt + 127) // 128)
        cores.append((src_c, dl))
    nbc = np.maximum(nbc, 1)
    nbs = nbc.sum(axis=1)                    # blocks per window
    offs = np.zeros(NW + 1, np.int64)        # slot offsets (units of slots)
    for w in range(NW):
        offs[w + 1] = offs[w] + nbs[w] * 128
    ts_sh = int(offs[-1])

    out = []
    for c in range(NCORES):
        src_c, dl = cores[c]
        w_of = dl // WSZ
        ch_of = (src_c % NS) // CH
        r1 = np.zeros(ts_sh, np.int64)       # within-chunk row ids, layer 0/1
        r2 = np.zeros(ts_sh, np.int64)
        s_tab = np.zeros((128, ts_sh), np.float32)   # [slot%128, col]
        st_tab = np.zeros((128, ts_sh), np.float32)  # [dst_loc, col]
        for w in range(NW):
            base = int(offs[w])
            for ch in range(NCH):
                mw = (w_of == w) & (ch_of == ch)
                cnt = int(mw.sum())
                i = np.arange(cnt)
                sw = src_c[mw]
                rid = (sw // NS) * CH + (sw % NS) % CH   # within-chunk row
                # sort slots by table row id: monotonic descriptor streams
                # are much friendlier to HBM than random order
                so = np.argsort(rid, kind="stable")
                rid = rid[so]
                dloc = (dl[mw] - w * WSZ)[so]            # 0..127
                r1[base + i] = rid
                r2[base + i] = rid
                s_tab[i % 128, base + (i // 128) * 128 + dloc] = 1.0
                st_tab[dloc, base + (i // 128) * 128 + (i % 128)] = 1.0
                base += int(nbc[w, ch]) * 128
        out.append({
            "isrc": _pack_idx(r1),
            "s": s_tab.astype(ml_dtypes.bfloat16),
            "st": st_tab.astype(ml_dtypes.bfloat16),
        })
    return out, nbc, offs, ts_sh


# --------------------------------------------------------------------------
# Tile assigns DMASW completion sems round-robin over all Pool DMA
# instructions, but the SWDGE ucode locks each sem to the queue that first
# uses it. Patch the lane choice to be queue-keyed: queue q owns lanes
# {2q, 2q+1}.
import concourse.tile_sem_assignment as _tsa


def _queue_keyed_assign_tick(self, inst):
    eng = inst.engine
    if (isinstance(inst, _tsa.DMAInst)
            and not isinstance(inst, _tsa.bass_isa.UserSyncedRemoteDMADescs)
            and eng == mybir.EngineType.Pool):
        q = int(getattr(inst, "queue_num", 0) or 0)
        cnt = getattr(self, "_per_q_cnt", None)
        if cnt is None:
            cnt = self._per_q_cnt = [0, 0, 0, 0]
        lane = (2 * q + cnt[q] % 2) % 8
        cnt[q] += 1
        self.next_sw_dma_idx = lane
        res = _tsa.TileClockTick._assign_tick_orig(self, inst)
        return res
    return _tsa.TileClockTick._assign_tick_orig(self, inst)


if not hasattr(_tsa.TileClockTick, "_assign_tick_orig"):
    _tsa.TileClockTick._assign_tick_orig = _tsa.TileClockTick._assign_tick
    _tsa.TileClockTick._assign_tick = _queue_keyed_assign_tick


# --------------------------------------------------------------------------
# device program
# --------------------------------------------------------------------------

def _build(nbc, offs, ts_sh, level: int = 0):
    nc = bacc.Bacc(None, target_bir_lowering=False, num_devices=NCORES,
                   num_swdge_queues=4)
    nbs = nbc.sum(axis=1)
    nb_max = int(max(nbs))

    xT = nc.dram_tensor("xT", [IN_DIM, NSP], F32, kind="ExternalInput")
    w1x = nc.dram_tensor("w1x", [IN_DIM, F0 + 8], BF16, kind="ExternalInput")
    b1r = nc.dram_tensor("b1r", [1, F0], F32, kind="ExternalInput")
    w2x = nc.dram_tensor("w2x", [F0, OUT + 2], BF16, kind="ExternalInput")
    b2r = nc.dram_tensor("b2r", [1, OUT], F32, kind="ExternalInput")
    stab = nc.dram_tensor("stab", [128, ts_sh], BF16, kind="ExternalInput")
    sttab = nc.dram_tensor("sttab", [128, ts_sh], BF16, kind="ExternalInput")
    isrc = nc.dram_tensor("isrc", [128, ts_sh // 16], I16, kind="ExternalInput")
    idb = nc.dram_tensor("idb", [128, 128], BF16, kind="ExternalInput")
    out = nc.dram_tensor("out", [NS, OUT], F32, kind="ExternalOutput")

    with tile.TileContext(nc) as tc:
        with (
            tc.tile_pool(name="const", bufs=1) as cp,
            tc.tile_pool(name="persist", bufs=1) as pp,
            tc.tile_pool(name="dram", bufs=1, space="DRAM") as dram,
        ):
            fe1_loc = [dram.tile([CH, FE1_W], BF16, name=f"fe1loc{c}")
                       for c in range(NCH)]
            fe2_loc = [dram.tile([CH, FE2_W], BF16, name=f"fe2loc{c}")
                       for c in range(NCH)]
            fe1_g = [dram.tile([NGC, FE1_W], BF16, addr_space="Shared",
                               name=f"fe1g{c}") for c in range(NCH)]
            fe2_g = [dram.tile([NGC, FE2_W], BF16, addr_space="Shared",
                               name=f"fe2g{c}") for c in range(NCH)]

            # -------- constants --------
            identb = cp.tile([128, 128], BF16)
            nc.sync.dma_start(identb[:], idb[:])
            w1sb = [cp.tile([128, F0 + 8], BF16, tag=f"w1_{k}", name=f"w1sb{k}")
                    for k in range(2)]
            for k in range(2):
                nc.sync.dma_start(w1sb[k][:], w1x[k * 128:(k + 1) * 128, :])
            w2sb = [cp.tile([128, OUT + 2], BF16, tag=f"w2_{k}", name=f"w2sb{k}")
                    for k in range(4)]
            for k in range(4):
                nc.sync.dma_start(w2sb[k][:], w2x[k * 128:(k + 1) * 128, :])
            b1one = cp.tile([1, F0], F32)
            nc.sync.dma_start(b1one[:], b1r[:])
            b1bc = cp.tile([128, F0], F32)
            nc.gpsimd.partition_broadcast(b1bc[:], b1one[:])
            b2one = cp.tile([1, OUT], F32)
            nc.sync.dma_start(b2one[:], b2r[:])
            b2bc = cp.tile([128, OUT], F32)
            nc.gpsimd.partition_broadcast(b2bc[:], b2one[:])
            i_t = cp.tile([128, ts_sh // 16], I16)
            nc.sync.dma_start(i_t[:], isrc[:])

            # x in bf16 (cast during DMA on the SWDGE path)
            xtsb = [pp.tile([128, NSP], BF16, tag=f"xt_{k}", name=f"xt{k}")
                    for k in range(2)]
            for k in range(2):
                nc.gpsimd.dma_start(xtsb[k][:], xT[k * 128:(k + 1) * 128, :])

            er1_sb = pp.tile([128, NW, H0], BF16, tag="er1_sb")
            er2_sb = pp.tile([128, NW, 1], BF16, tag="er2_sb")
            hT = [pp.tile([128, NSP], BF16, tag=f"hT_{f}", name=f"hT{f}")
                  for f in range(4)]

            grp = [list(range(NCORES))]
            WPC = NW // NCH                      # windows per chunk (10)

            # ============= Phase A: layer-0 node compute =============
            with (
                tc.tile_pool(name="pA", bufs=2, space="PSUM") as pA,
                tc.tile_pool(name="sA", bufs=3) as sA,
            ):
                for m in range(NW):
                    mc = slice(m * 128, (m + 1) * 128)
                    ps = pA.tile([128, F0], F32, tag="ft")
                    for k in range(2):
                        nc.tensor.matmul(ps[:], xtsb[k][:, mc],
                                         w1sb[k][:, 0:F0],
                                         start=(k == 0), stop=(k == 1))
                    pse = pA.tile([128, 8], F32, tag="elr")
                    for k in range(2):
                        nc.tensor.matmul(pse[:], xtsb[k][:, mc],
                                         w1sb[k][:, F0:F0 + 8],
                                         start=(k == 0), stop=(k == 1))
                    fem = sA.tile([128, 516], BF16, tag="fem")
                    if m % 2 == 0:
                        nc.vector.tensor_copy(fem[:, 0:F0], ps[:])
                    else:
                        nc.scalar.copy(fem[:, 0:F0], ps[:])
                    nc.scalar.copy(fem[:, F0:F0 + 4], pse[:, 0:4])
                    nc.vector.tensor_copy(er1_sb[:, m, :], pse[:, 4:8])
                    ci, lo = m // WPC, (m % WPC) * 128
                    nc.sync.dma_start(fe1_loc[ci][lo:lo + 128, 0:516], fem[:])
                    if m % WPC == WPC - 1:
                        nc.gpsimd.collective_compute(
                            "AllGather", OP.bypass, grp,
                            ins=[fe1_loc[ci][:].opt()],
                            outs=[fe1_g[ci][:].opt()])

            if level == 1:
                with tc.tile_pool(name="dbg", bufs=2) as db:
                    for m in range(NW):
                        lo, hi = m * 128, min((m + 1) * 128, NS)
                        t = db.tile([128, OUT], BF16, tag="d")
                        ci, clo = m // WPC, (m % WPC) * 128
                        nc.sync.dma_start(
                            t[0:hi - lo, :],
                            fe1_loc[ci][clo:clo + hi - lo, 0:OUT])
                        t2 = db.tile([128, OUT], F32, tag="d2")
                        nc.vector.tensor_copy(t2[0:hi - lo, :], t[0:hi - lo, :])
                        nc.sync.dma_start(out[lo:hi, :], t2[0:hi - lo, :])

            # ============= edge aggregation =============
            def edge_layer(fe_g, er_sb, fe_w, nhead, dfeat, finalize, lnum):
                nhf = nhead * dfeat                      # 512 / 64
                mw = nhf + 16                            # msg row incl ex+pad
                with (
                    tc.tile_pool(name=f"pB{lnum}", bufs=2, space="PSUM") as pB,
                    tc.tile_pool(name=f"pBs{lnum}", bufs=1, space="PSUM") as pBs,
                    tc.tile_pool(name=f"sB{lnum}", bufs=2) as sB,
                    tc.tile_pool(name=f"sB3{lnum}", bufs=2) as sB3,
                ):
                    # absorb table sems into the POOL engine clock so the
                    # per-window gathers don't each carry a blocking wait
                    for ch in range(NCH):
                        dmyb = sB.tile([1, 16], BF16, tag=f"dmyb{ch}")
                        nc.gpsimd.dma_start(dmyb[:], fe_g[ch][0:1, 0:16])

                    for w in range(NW):
                        nb = int(nbs[w])
                        off = int(offs[w])               # slot offset
                        feg = sB.tile([128, nb_max, fe_w], BF16, tag="feg")
                        b0 = 0
                        qn = 0
                        for ch in range(NCH):
                            bn = int(nbc[w, ch])
                            # split into <=5-block calls: smaller calls keep
                            # the SWDGE ring from backing up the POOL engine
                            segs = []
                            s0 = 0
                            seg_max = 4 if fe_w > 256 else 5
                            while s0 < bn:
                                s1 = min(s0 + seg_max, bn)
                                segs.append((s0, s1))
                                s0 = s1
                            for (s0_, s1_) in segs:
                                qcol = slice(off // 16 + (b0 + s0_) * 8,
                                             off // 16 + (b0 + s1_) * 8)
                                nc.gpsimd.dma_gather(
                                    feg[:, b0 + s0_:b0 + s1_, :], fe_g[ch][:],
                                    i_t[:, qcol],
                                    num_idxs=(s1_ - s0_) * 128,
                                    num_idxs_reg=(s1_ - s0_) * 128,
                                    elem_size=fe_w, single_packet=True,
                                    queue_num=(4 * w + qn) % 4)
                                qn += 1
                            b0 += bn
                        cs = slice(off, off + nb * 128)
                        s_t = sB.tile([128, nb_max * 128], BF16, tag="s_t")
                        nc.sync.dma_start(s_t[:, 0:nb * 128], stab[:, cs])
                        st_t = sB.tile([128, nb_max * 128], BF16, tag="st_t")
                        nc.sync.dma_start(st_t[:, 0:nb * 128], sttab[:, cs])

                        # er expansion: er_e[slot, h] via S^T_b @ er_win
                        er_ps = pB.tile([128, nb_max * nhead], F32, tag="er_ps")
                        for b in range(nb):
                            bs = slice(b * 128, (b + 1) * 128)
                            nc.tensor.matmul(
                                er_ps[:, b * nhead:(b + 1) * nhead],
                                st_t[:, bs], er_sb[:, w, :],
                                start=True, stop=True)

                        # logits: leaky_relu = max(x,0) + 0.2*min(x,0)
                        epre = sB.tile([128, nb_max * nhead], F32, tag="epre")
                        nc.vector.tensor_tensor(
                            out=epre[:, 0:nb * nhead],
                            in0=feg[:, 0:nb, nhf:nhf + nhead],
                            in1=er_ps[:, 0:nb * nhead].rearrange(
                                "p (b h) -> p b h", h=nhead),
                            op=OP.add)
                        t04 = sB.tile([128, nb_max * nhead], F32, tag="t04")
                        nc.vector.tensor_scalar(
                            out=t04[:, 0:nb * nhead],
                            in0=epre[:, 0:nb * nhead],
                            scalar1=0.0, scalar2=0.2,
                            op0=OP.min, op1=OP.mult)
                        nc.vector.scalar_tensor_tensor(
                            out=epre[:, 0:nb * nhead],
                            in0=epre[:, 0:nb * nhead],
                            scalar=0.0, in1=t04[:, 0:nb * nhead],
                            op0=OP.max, op1=OP.add)
                        # exp + broadcast-expand in one ACT op: exe[p,b,h,d]
                        # = exp(pre[p,b,h]) for every d (0-stride INPUT view,
                        # dense output -> downstream DVE stays in 2x mode)
                        exe = sB3.tile([128, nb_max, nhf], BF16, tag="exe")
                        nc.scalar.activation(
                            exe[:, 0:nb, :].rearrange(
                                "p b (h d) -> p b h d", d=dfeat),
                            epre[:, 0:nb * nhead].rearrange(
                                "p (b h) -> p b h", h=nhead).broadcast_to(
                                (128, nb, nhead, dfeat)),
                            AF.Exp)
                        # msg = feat * ex  (dense x dense, 2x mode)
                        msg = sB3.tile([128, nb_max, nhf], BF16, tag="msg")
                        nc.vector.tensor_tensor(
                            out=msg[:, 0:nb, :],
                            in0=feg[:, 0:nb, 0:nhf],
                            in1=exe[:, 0:nb, :],
                            op=OP.mult)

                        rst = pB.tile([128, nhf], F32, tag="rst")
                        spsT = pB.tile([nhead, 128], F32, tag="spsT")
                        for b in range(nb):
                            bs = slice(b * 128, (b + 1) * 128)
                            nc.tensor.matmul(rst[:], s_t[:, bs],
                                             msg[:, b, :],
                                             start=(b == 0), stop=(b == nb - 1))
                            nc.tensor.matmul(
                                spsT[:],
                                exe[:, b, :].rearrange(
                                    "p (h d) -> p h d", d=dfeat)[:, :, 0:1],
                                s_t[:, bs],
                                start=(b == 0), stop=(b == nb - 1),
                                skip_group_check=True)
                        spsb = sB.tile([nhead, 128], BF16, tag="spsb")
                        nc.vector.tensor_copy(spsb[:], spsT[:])
                        spt = pBs.tile([128, nhead], BF16, tag="spt")
                        nc.tensor.transpose(spt[:], spsb[:],
                                            identb[0:nhead, 0:nhead])
                        ssb = sB.tile([128, nhead], F32, tag="ssb")
                        nc.vector.tensor_scalar(out=ssb[:], in0=spt[:],
                                                scalar1=1e-30, scalar2=None,
                                                op0=OP.max)
                        rec = sB.tile([128, nhead], F32, tag="rec")
                        nc.vector.reciprocal(rec[:], ssb[:])
                        finalize(w, rst, rec, pBs, sB)

            # ---- layer 0 finalize: 1/s, +b1, ELU, transpose into hT ----
            def fin0(w, rst, rec, pF, sB):
                mc = slice(w * 128, (w + 1) * 128)
                hsb = sB.tile([128, F0], F32, tag="hsb")
                nc.vector.tensor_tensor(
                    out=hsb[:].rearrange("p (h d) -> p h d", d=D0),
                    in0=rst[:].rearrange("p (h d) -> p h d", d=D0),
                    in1=rec[:].broadcast_to((128, H0, D0)),
                    op=OP.mult)
                nc.vector.tensor_tensor(out=hsb[:], in0=hsb[:], in1=b1bc[:],
                                        op=OP.add)
                # ELU(x) = (max(x,0)-1) + exp(min(x,0))
                pos = sB.tile([128, F0], F32, tag="pos")
                nc.vector.tensor_scalar(out=pos[:], in0=hsb[:], scalar1=0.0,
                                        scalar2=-1.0, op0=OP.max, op1=OP.add)
                expn = sB.tile([128, F0], F32, tag="expn")
                nc.scalar.activation(expn[:], hsb[:], AF.Exp)
                nc.vector.tensor_scalar(out=expn[:], in0=expn[:], scalar1=1.0,
                                        scalar2=None, op0=OP.min)
                heb = sB.tile([128, F0], BF16, tag="heb")
                nc.vector.tensor_tensor(out=heb[:], in0=pos[:], in1=expn[:],
                                        op=OP.add)
                for f in range(4):
                    pt = pF.tile([128, 128], BF16, tag="tp0")
                    nc.tensor.transpose(pt[:], heb[:, f * 128:(f + 1) * 128],
                                        identb[:])
                    if f % 2 == 0:
                        nc.vector.tensor_copy(hT[f][:, mc], pt[:])
                    else:
                        nc.scalar.copy(hT[f][:, mc], pt[:])

            if level == 0 or level >= 2:
                edge_layer(fe1_g, er1_sb, FE1_W, H0, D0, fin0, 0)

            if level == 2:
                with tc.tile_pool(name="dbg2", bufs=2) as db:
                    for m in range(NW):
                        lo, hi = m * 128, min((m + 1) * 128, NS)
                        t2 = db.tile([128, OUT], F32, tag="d2")
                        nc.vector.tensor_copy(
                            t2[0:hi - lo, :],
                            hT[0][:, m * 128:m * 128 + hi - lo]
                            [0:hi - lo, 0:OUT])
                        nc.sync.dma_start(out[lo:hi, :], t2[0:hi - lo, :])

            # ============= transition: layer-1 node compute =============
            if level == 0 or level >= 3:
                with (
                    tc.tile_pool(name="pT", bufs=2, space="PSUM") as pT,
                    tc.tile_pool(name="sT", bufs=3) as sT,
                ):
                    for m in range(NW):
                        mc = slice(m * 128, (m + 1) * 128)
                        ps2 = pT.tile([128, OUT], F32, tag="f2")
                        for k in range(4):
                            nc.tensor.matmul(ps2[:], hT[k][:, mc],
                                             w2sb[k][:, 0:OUT],
                                             start=(k == 0), stop=(k == 3))
                        pse2 = pT.tile([128, 2], F32, tag="el2")
                        for k in range(4):
                            nc.tensor.matmul(pse2[:], hT[k][:, mc],
                                             w2sb[k][:, OUT:OUT + 2],
                                             start=(k == 0), stop=(k == 3))
                        fem2 = sT.tile([128, 65], BF16, tag="fem2")
                        nc.vector.tensor_copy(fem2[:, 0:OUT], ps2[:])
                        nc.scalar.copy(fem2[:, OUT:OUT + 1], pse2[:, 0:1])
                        nc.vector.tensor_copy(er2_sb[:, m, :], pse2[:, 1:2])
                        ci, lo = m // WPC, (m % WPC) * 128
                        nc.sync.dma_start(fe2_loc[ci][lo:lo + 128, 0:65],
                                          fem2[:])
                        if m % WPC == WPC - 1:
                            nc.gpsimd.collective_compute(
                                "AllGather", OP.bypass, grp,
                                ins=[fe2_loc[ci][:].opt()],
                                outs=[fe2_g[ci][:].opt()])

                # ---- layer 1 finalize: 1/s, +b2, write output rows ----
                def fin1(w, rst, rec, pF, sB):
                    osb = sB.tile([128, OUT], F32, tag="osb")
                    nc.vector.tensor_scalar(out=osb[:], in0=rst[:],
                                            scalar1=rec[:, 0:1],
                                            scalar2=None, op0=OP.mult)
                    nc.vector.tensor_tensor(out=osb[:], in0=osb[:],
                                            in1=b2bc[:], op=OP.add)
                    lo, hi = w * 128, min((w + 1) * 128, NS)
                    nc.sync.dma_start(out[lo:hi, :], osb[0:hi - lo, :])

                if level == 0 or level >= 4:
                    edge_layer(fe2_g, er2_sb, FE2_W, 1, OUT, fin1, 1)

    nc.compile()
    return nc


# --------------------------------------------------------------------------
# entry point
# --------------------------------------------------------------------------

LAST_RESULTS = None


def kernel(x, src, dst, W1, al1, ar1, b1, W2, al2, ar2, b2):
    x = np.asarray(x, np.float32)
    W1 = np.asarray(W1, np.float32)
    W2 = np.asarray(W2, np.float32)
    al1 = np.asarray(al1, np.float32)
    ar1 = np.asarray(ar1, np.float32)
    al2 = np.asarray(al2, np.float32)
    ar2 = np.asarray(ar2, np.float32)
    b1 = np.asarray(b1, np.float32)
    b2 = np.asarray(b2, np.float32)

    per_core, nbc, offs, ts_sh = _host_prep(src, dst)
    nc = _build(nbc, offs, ts_sh, level=int(os.environ.get('K_LEVEL', '0')))

    # fold attention vectors into extra weight columns (placement + small
    # deterministic precompute on O(param) data)
    w1xx = np.zeros((IN_DIM, F0 + 8), np.float32)
    w1xx[:, 0:F0] = W1
    for h in range(H0):
        w1xx[:, F0 + h] = W1[:, h * D0:(h + 1) * D0] @ al1[h]
        w1xx[:, F0 + 4 + h] = W1[:, h * D0:(h + 1) * D0] @ ar1[h]
    w2xx = np.zeros((F0, OUT + 2), np.float32)
    w2xx[:, 0:OUT] = W2
    w2xx[:, OUT] = W2 @ al2[0]
    w2xx[:, OUT + 1] = W2 @ ar2[0]
    identb = np.eye(128, dtype=ml_dtypes.bfloat16)

    in_maps = []
    for c in range(NCORES):
        xc = np.zeros((IN_DIM, NSP), np.float32)
        xc[:, :NS] = x[c * NS:(c + 1) * NS].T
        in_maps.append({
            "xT": np.ascontiguousarray(xc),
            "w1x": w1xx.astype(ml_dtypes.bfloat16),
            "b1r": b1.reshape(1, F0).copy(),
            "w2x": w2xx.astype(ml_dtypes.bfloat16),
            "b2r": b2.reshape(1, OUT).copy(),
            "stab": per_core[c]["s"],
            "sttab": per_core[c]["st"],
            "isrc": per_core[c]["isrc"],
            "idb": identb,
        })

    trace = bool(int(os.environ.get("K_TRACE", "0")))
    res = run_bass_kernel_spmd(nc, in_maps, core_ids=list(range(NCORES)),
                               trace=trace)
    global LAST_RESULTS
    LAST_RESULTS = res
    return np.concatenate([res.results[c]["out"] for c in range(NCORES)],
                          axis=0)
